# revision 101
# baseline (speedup 1.0000x reference)
"""BiBloSAN Trainium2 kernel — rank-2 separable softmax approximation.

Shapes: B=4, N=16 blocks, R=64 tokens/block, D=256.
Sharding: one (batch, direction) pair per core -> 8 cores, no collectives.
The bw direction runs the SAME SPMD program on a host-reversed token
sequence (flat reverse maps the j<i mask onto the j>i program exactly).

Intra-block mSA approximation: the pairwise weight
    g(u) = exp(C*tanh(u/C)),  u = xi[i,d] + xj[j,d] + b[d]
is replaced by a 2-term exponential fit
    g(u) ~= c1 e^{s u} + c2 e^{2 s u}
tuned END-TO-END against the exact reference (max rel err 3.8e-3 in a
bit-accurate numpy mirror; gate is 2e-2).  Each term is separable:
e^{ksu} = (zh wh)^{2k} with zh = e^{(s/2)(xjb-SH)}, wh = e^{(s/2)(xi+SH)},
so the masked-softmax num/den become per-block suffix sums of zh-powers
(triangular matmuls, c_k folded into the stationary).  The common factor
wh^2 cancels in num/den, so the recombination is a single Horner step:
    num|den = (wh^2 ⊙ S2) + S1,   h = num/den
where S1 = c1·tri @ [z^2 x | z^2] (den stationary carries an extra
diagonal at the last row of each block so empty rows give h=0), and
S2 = c2·tri @ [z^4 x | z^4].

s2t block summaries are computed token-major so the per-block softmax
sums become matmuls against block-indicator stationaries (no DVE
reductions).  Sigmoids are rewritten as 0.5+0.5*tanh(z/2) to stay on the
exp/tanh/relu activation table (no table reloads).
"""

import numpy as np
from contextlib import ExitStack

import concourse.bass as bass
import concourse.mybir as mybir
import concourse.tile as tile
from concourse import bacc, bass_utils

F32 = mybir.dt.float32
F16 = mybir.dt.float16
AF = mybir.ActivationFunctionType
ALU = mybir.AluOpType

B, NB, R, D = 4, 16, 64, 256
T = NB * R          # 1024 tokens
DT = D // 128       # 2 partition tiles of feature dim
NCORES = 8
NTILE = T // 128    # 8 token tiles (2 blocks each)

# end-to-end tuned rank-2 fit of exp(5*tanh(u/5)):
#   g(u) ~= C1 e^{S u} + C2 e^{2 S u}
SFIT = 0.97664077
C1 = 0.76476878
C2 = -0.00151352
SHIFT = 2.0
S2F = SFIT / 2.0
BZ = -S2F * SHIFT   # zh = exp(S2F*xjb + BZ)
BW = SFIT * SHIFT   # w2 = exp(SFIT*xi + BW)

# software-pipeline schedule (stage offsets in us-equivalents, quarter pitch)
SCHED_OFFS = (0.0, 1.065, 2.047, 2.331, 2.862, 2.883, 4.286, 4.45,
              5.321, 6.457, 6.833, 7.519)
SCHED_DQ = 0.95
WORK_BUFS = 4
PGEM_BUFS = 3
PSP_BUFS = 2
PSML_BUFS = 3

# f16 pack column offsets
PH = {}
_c = 0
def _ph(nm, w):
    global _c
    PH[nm] = _c
    _c += w
_ph("bias", 30)     # f32 per-partition biases, bitcast into the f16 pack
_ph("fcW", 512)
_ph("fcbT", D)
_ph("idm", 128)
_ph("mW2", 512)
_ph("mb_row", D)
NPK1 = _c           # DMA chunk 1: needed by stages A/B(xjb)
_ph("triC1", 128)
_ph("triC1E", 128)
_ph("triC2", 128)
_ph("mW1", 512)
NPK2 = _c           # DMA chunk 2: stages B-E
_ph("bk0", 4)       # block indicator, tile 0 of quarter
_ph("bk1", 4)
_ph("mask0", NB)
_ph("ones_row", 256)
_ph("s2tb_row", D)
_ph("s2tW1", 512)
_ph("s2tW", 512)
NPK3 = _c           # DMA chunk 3: stages G-I
_ph("gW1", 512)
_ph("gW2", 512)
_ph("fW1", 1536)
_ph("fW2", 1536)
NPACKH = _c

# f32 per-partition bias columns (feature-major, DT cols each) inside the
# bitcast "bias" block of the f16 pack
PB = {"fcb": 0, "s2tb1": 2, "gbh": 4, "fb1h": 6, "fb2h": 8, "mbf": 10}
P_BZ, P_BW, P_BZ4 = 12, 13, 14  # broadcast scalar biases for the exps


def _ap(t, offset, dims):
    """Raw AP on sbuf/psum tile t: dims = [[step, count], ...] free dims."""
    base = t[:]
    return bass.AP(tensor=base.tensor, offset=base.offset + offset,
                   ap=[list(base.ap[0])] + [list(d) for d in dims])


def build_nc():
    nc = bacc.Bacc("TRN2", target_bir_lowering=False, debug=False,
                   num_devices=NCORES)

    xT_d = nc.dram_tensor("xT", [D, T], F16, kind="ExternalInput").ap()
    packh_d = nc.dram_tensor("packf16", [128, NPACKH], F16,
                             kind="ExternalInput").ap()
    out_d = nc.dram_tensor("outT", [D, 32], F16, kind="ExternalOutput").ap()

    with tile.TileContext(nc) as tc, ExitStack() as ctx:
        ctx.enter_context(nc.allow_low_precision(
            reason="f16 softmax pipeline validated end-to-end vs reference"))
        # noqa: engine split: Act=exps/relus (PSUM-fed), DVE=PSUM-touching
        # muls/recips, Pool(gpsimd)=SBUF-only muls, PE=GEMMs+suffix-sums
        const = ctx.enter_context(tc.tile_pool(name="const", bufs=1))
        big = ctx.enter_context(tc.tile_pool(name="big", bufs=1))
        work = ctx.enter_context(tc.tile_pool(name="work", bufs=WORK_BUFS))
        pgem = ctx.enter_context(
            tc.tile_pool(name="pgem", bufs=PGEM_BUFS, space="PSUM"))
        psp = ctx.enter_context(
            tc.tile_pool(name="psp", bufs=PSP_BUFS, space="PSUM"))
        psml = ctx.enter_context(
            tc.tile_pool(name="psml", bufs=PSML_BUFS, space="PSUM"))

        # ---- DMA loads, ordered by consumer stage ----
        pkh = const.tile([128, NPACKH], F16, tag="packh")
        xT = big.tile([128, DT, T], F16, tag="xT")
        nc.sync.dma_start(out=pkh[:, 0:NPK1], in_=packh_d[:, 0:NPK1])
        nc.sync.dma_start(
            out=_ap(xT, 0, [[T, DT], [1, 512]]),
            in_=bass.AP(tensor=xT_d.tensor, offset=0,
                        ap=[[T, 128], [128 * T, DT], [1, 512]]))
        nc.sync.dma_start(out=pkh[:, NPK1:NPK2], in_=packh_d[:, NPK1:NPK2])
        nc.sync.dma_start(
            out=_ap(xT, 512, [[T, DT], [1, 512]]),
            in_=bass.AP(tensor=xT_d.tensor, offset=512,
                        ap=[[T, 128], [128 * T, DT], [1, 512]]))
        nc.sync.dma_start(out=pkh[:, NPK2:NPK3], in_=packh_d[:, NPK2:NPK3])
        nc.sync.dma_start(out=pkh[:, NPK3:], in_=packh_d[:, NPK3:])

        wp = {nm: pkh[:, c:c + 512].rearrange("p (kt e) -> p kt e", kt=DT)
              for nm, c in PH.items()
              if nm in ("fcW", "mW1", "mW2", "s2tW1", "s2tW", "gW1", "gW2")}
        wp.update({nm: pkh[:, PH[nm]:PH[nm] + 1536].rearrange(
            "p (kt e) -> p kt e", kt=6) for nm in ("fW1", "fW2")})
        triC1 = pkh[:, PH["triC1"]:PH["triC1"] + 128]
        triC1E = pkh[:, PH["triC1E"]:PH["triC1E"] + 128]
        triC2 = pkh[:, PH["triC2"]:PH["triC2"] + 128]
        idm = pkh[:, PH["idm"]:PH["idm"] + 128]
        bk = [pkh[:, PH["bk0"]:PH["bk0"] + 4], pkh[:, PH["bk1"]:PH["bk1"] + 4]]
        mask0 = pkh[:, PH["mask0"]:PH["mask0"] + NB]
        ones_row = pkh[0:1, PH["ones_row"]:PH["ones_row"] + 128]
        ones_row256 = pkh[0:1, PH["ones_row"]:PH["ones_row"] + 256]
        mb_row = pkh[0:1, PH["mb_row"]:PH["mb_row"] + D]
        s2tb_row = pkh[0:1, PH["s2tb_row"]:PH["s2tb_row"] + D]
        bsb = {nm: pkh[:, 2 * c:2 * (c + DT)].bitcast(F32)
               for nm, c in PB.items()}

        # dummy activation to hoist the exp-table load off the critical path
        wrm = const.tile([1, 2], F32, tag="wrm")
        nc.vector.memset(wrm[:], 0.0)
        nc.scalar.activation(wrm[:, 1:2], wrm[:, 0:1], AF.Exp)

        inp = big.tile([128, DT, T], F16, tag="inp")
        h_tok = big.tile([128, NTILE, D], F16, tag="h_tok")
        hT = big.tile([128, DT, T], F16, tag="hT")
        v_sb = big.tile([4, 4, D], F16, tag="v_sb")
        vT = const.tile([128, DT, NB], F16, tag="vT")
        vi_sb = const.tile([128, DT, 1], F32, tag="vi_sb")
        u0 = const.tile([128, DT, NB], F32, tag="u0")
        g0 = const.tile([128, DT, NB], F16, tag="g0")
        wv = const.tile([128, DT, NB], F16, tag="wv")
        ndp = const.tile([128, DT, 4, 2], F32, tag="ndp")
        qs = [dict() for _ in range(4)]

        # The engines execute their queues IN PROGRAM ORDER, so the quarters
        # are emitted as software-pipelined stages, interleaved by expected
        # start time; otherwise quarter q+1's ready work would sit blocked
        # behind quarter q's unfinished chain in every engine queue.
        def stA(q):  # P1 chunk (feature-major FC) + inpH via transposes
            tok0 = q * 256
            p1 = pgem.tile([128, DT, 256], F32, tag="gem", name="p1")
            for mt in range(DT):
                for kt in range(DT):
                    nc.tensor.matmul(
                        p1[:, mt, :],
                        wp["fcW"][:, kt, mt * 128:(mt + 1) * 128],
                        xT[:, kt, tok0:tok0 + 256],
                        start=(kt == 0), stop=False)
                nc.tensor.matmul(
                    p1[:, mt, :],
                    pkh[0:1, PH["fcbT"] + mt * 128:PH["fcbT"] + mt * 128 + 128],
                    ones_row256, start=False, stop=True)
            nc.scalar.activation(
                _ap(inp, tok0, [[T, DT], [1, 256]]), p1[:], AF.Relu)
            ptr2 = psml.tile([128, 2, DT, 128], F16, tag="sml", name="ptr2")
            for t in range(2):
                tk = tok0 + t * 128
                for dt in range(DT):
                    nc.tensor.transpose(ptr2[:, t, dt, :],
                                        inp[:, dt, tk:tk + 128], idm)
            qs[q]["ptr2"] = ptr2

        def stB(q):  # xi/xjb GEMMs
            tok0 = q * 256
            pxi = pgem.tile([128, 2, D], F32, tag="gem", name="pxi")
            pxj = pgem.tile([128, 2, D], F32, tag="gem", name="pxj")
            for t in range(2):
                tk = tok0 + t * 128
                for kt in range(DT):
                    nc.tensor.matmul(pxj[:, t, :], inp[:, kt, tk:tk + 128],
                                     wp["mW2"][:, kt, :],
                                     start=(kt == 0), stop=False)
                nc.tensor.matmul(pxj[:, t, :], ones_row, mb_row,
                                 start=False, stop=True)
                for kt in range(DT):
                    nc.tensor.matmul(pxi[:, t, :], inp[:, kt, tk:tk + 128],
                                     wp["mW1"][:, kt, :],
                                     start=(kt == 0), stop=(kt == DT - 1))
            qs[q]["pxi"], qs[q]["pxj"] = pxi, pxj

        def stC(q):  # exps: z2 (from pxj), w2 (from pxi)
            pxi, pxj = qs[q]["pxi"], qs[q]["pxj"]
            w2 = work.tile([128, 2, D], F32, tag="w2", name="w2")
            zall = work.tile([128, 2, 2, 2, D], F16, tag="zall", name="zall")
            nc.scalar.activation(
                _ap(zall, 2 * D, [[4 * D, 2], [1, D]]), pxj[:],
                AF.Exp, scale=SFIT,
                bias=pkh[:, 2 * P_BZ:2 * P_BZ + 2].bitcast(F32))    # z2
            nc.scalar.activation(w2[:], pxi[:], AF.Exp, scale=SFIT,
                                 bias=pkh[:, 2 * P_BW:2 * P_BW + 2]
                                 .bitcast(F32))
            qs[q]["w2"], qs[q]["zall"] = w2, zall

        def stD(q):  # z powers: zall[p,t,xp,k,d]: xp=0 -> z^k*x, xp=1 -> z^k
            zall = qs[q]["zall"]
            ptr2 = qs[q]["ptr2"]
            inpH_b = _ap(ptr2, 0, [[D, 2], [1, D]])
            nc.gpsimd.tensor_tensor(
                _ap(zall, 3 * D, [[4 * D, 2], [1, D]]),
                _ap(zall, 2 * D, [[4 * D, 2], [1, D]]),
                _ap(zall, 2 * D, [[4 * D, 2], [1, D]]), ALU.mult)      # z4
            nc.vector.tensor_mul(
                _ap(zall, 0, [[4 * D, 2], [1, D]]),
                _ap(zall, 2 * D, [[4 * D, 2], [1, D]]),
                inpH_b)                                                # z2*x
            nc.vector.tensor_mul(
                _ap(zall, 1 * D, [[4 * D, 2], [1, D]]),
                _ap(zall, 3 * D, [[4 * D, 2], [1, D]]), inpH_b)       # z4*x

        def stE1(q):  # S2 suffix-sum matmuls + in-place w2 mul
            zall, w2 = qs[q]["zall"], qs[q]["w2"]
            Ss = []
            for t in range(2):
                S = psp.tile([128, 2 * D], F32, tag="S", name="S")
                nc.tensor.matmul(S[:], triC2,
                                 _ap(zall, t * 4 * D + D,
                                     [[2 * D, 2], [1, D]]),
                                 start=True, stop=True)
                Ss.append(S)
            for t in range(2):
                w2_b = _ap(w2, t * D, [[0, 2], [1, D]])
                nc.vector.tensor_mul(Ss[t][:], Ss[t][:], w2_b)
            qs[q]["Ss"] = Ss

        def stE2(q):  # S1 matmuls ACCUMULATE on top in PSUM, then h = n/d
            zall, Ss = qs[q]["zall"], qs[q]["Ss"]
            for t in range(2):
                nc.tensor.matmul(Ss[t][:, 0:D], triC1,
                                 _ap(zall, t * 4 * D, [[1, D]]),
                                 start=False, stop=True,
                                 skip_group_check=True)
                nc.tensor.matmul(Ss[t][:, D:2 * D], triC1E,
                                 _ap(zall, t * 4 * D + 2 * D, [[1, D]]),
                                 start=False, stop=True,
                                 skip_group_check=True)
            rden = work.tile([128, 2, D], F32, tag="rden", name="rden")
            for t in range(2):
                nc.vector.reciprocal(rden[:, t, :], Ss[t][:, D:2 * D])
                nc.vector.tensor_mul(h_tok[:, 2 * q + t, :], Ss[t][:, 0:D],
                                     rden[:, t, :])

        def stF(q):  # transpose h to feature-major
            for t in range(2):
                ptr = psml.tile([128, DT, 128], F16, tag="sml", name="ptr")
                for dt in range(DT):
                    nc.tensor.transpose(
                        ptr[:, dt, :],
                        h_tok[:, 2 * q + t, dt * 128:(dt + 1) * 128], idm)
                hT_dst = _ap(hT, (2 * q + t) * 128, [[T, DT], [1, 128]])
                nc.scalar.activation(hT_dst, ptr[:], AF.Copy)

        def stG(q):  # s2t stage 1: f = relu(hT.T@W1 + b1), feature-major
            tok0 = q * 256
            pf = psml.tile([128, DT, 256], F32, tag="sml", name="pf")
            for t in range(2):
                for mt in range(DT):
                    for kt in range(DT):
                        nc.tensor.matmul(
                            pf[:, mt, t * 128:(t + 1) * 128],
                            wp["s2tW1"][:, kt, mt * 128:(mt + 1) * 128],
                            hT[:, kt, tok0 + t * 128:tok0 + (t + 1) * 128],
                            start=(kt == 0), stop=(kt == DT - 1))
            fTq = work.tile([128, DT, 256], F16, tag="fTq", name="fTq")
            for mt in range(DT):
                nc.scalar.activation(fTq[:, mt, :], pf[:, mt, :], AF.Relu,
                                     bias=bsb["s2tb1"][:, mt:mt + 1])
            qs[q]["fTq"] = fTq

        def stH(q):  # s2t stage 2: e = exp(f@W + b), token-major
            fTq = qs[q]["fTq"]
            end = work.tile([128, 2, 2, D], F16, tag="end", name="end")
            pe = psml.tile([128, 2, D], F32, tag="sml", name="pe")
            for t in range(2):
                for mt in range(DT):
                    nc.tensor.matmul(pe[:, t, :],
                                     fTq[:, mt, t * 128:(t + 1) * 128],
                                     wp["s2tW"][:, mt, :],
                                     start=(mt == 0), stop=False)
                nc.tensor.matmul(pe[:, t, :], ones_row, s2tb_row,
                                 start=False, stop=True)
            nc.scalar.activation(_ap(end, 0, [[2 * D, 2], [1, D]]), pe[:],
                                 AF.Exp)
            qs[q]["end"] = end

        def stI(q):  # block sums via indicator matmuls -> v rows
            end = qs[q]["end"]
            vq = psml.tile([4, 2 * D], F32, tag="sml", name="vq")
            for t in range(2):
                nc.gpsimd.tensor_tensor(end[:, t, 1, :], end[:, t, 0, :],
                                        h_tok[:, 2 * q + t, :], ALU.mult)
                nc.tensor.matmul(vq[:], bk[t], end[:, t, :, :],
                                 start=(t == 0), stop=(t == 1))
            rdv = work.tile([4, D], F32, tag="rdv", name="rdv")
            nc.vector.reciprocal(rdv[:], vq[:, 0:D])
            nc.vector.tensor_mul(v_sb[:, q, :], vq[:, D:2 * D], rdv[:])

        def stJ(q):  # transpose this quarter's 4 v rows into vT columns
            ptrV = psml.tile([128, DT, 4], F16, tag="sml", name="ptrV")
            for dt in range(DT):
                nc.tensor.transpose(
                    ptrV[:, dt, :], v_sb[:, q, dt * 128:(dt + 1) * 128],
                    pkh[0:4, PH["idm"]:PH["idm"] + 4])
            nc.vector.tensor_copy(vT[:, :, 4 * q:4 * q + 4], ptrV[:])

        def stK(q):  # block-mSA row-0 partials for this quarter's 4 columns
            pvj = psml.tile([128, DT, 4], F32, tag="sml", name="pvj")
            if q == 0:
                pvi = psml.tile([128, DT, 1], F32, tag="sml", name="pvi")
                for mt in range(DT):
                    for kt in range(DT):
                        nc.tensor.matmul(
                            pvi[:, mt, :],
                            wp["mW1"][:, kt, mt * 128:(mt + 1) * 128],
                            vT[:, kt, 0:1], start=(kt == 0),
                            stop=(kt == DT - 1))
                nc.vector.tensor_copy(vi_sb[:], pvi[:])
            for mt in range(DT):
                for kt in range(DT):
                    nc.tensor.matmul(
                        pvj[:, mt, :],
                        wp["mW2"][:, kt, mt * 128:(mt + 1) * 128],
                        vT[:, kt, 4 * q:4 * q + 4], start=(kt == 0),
                        stop=(kt == DT - 1))
            for mt in range(DT):
                nc.vector.scalar_tensor_tensor(
                    u0[:, mt, 4 * q:4 * q + 4], pvj[:, mt, :],
                    bsb["mbf"][:, mt:mt + 1],
                    _ap(vi_sb, mt, [[0, 4]]), ALU.add, ALU.add)
            u0q = _ap(u0, 4 * q, [[NB, DT], [1, 4]])
            g0q = _ap(g0, 4 * q, [[NB, DT], [1, 4]])
            nc.scalar.activation(u0q, u0q, AF.Tanh, scale=0.2)
            nc.scalar.activation(g0q, u0q, AF.Exp, scale=5.0)
            if q == 0:
                nc.vector.memset(_ap(g0, 0, [[NB, DT], [1, 1]]), 0.0)
            nc.vector.tensor_reduce(ndp[:, :, q, 0], g0q,
                                    mybir.AxisListType.X, ALU.add)
            nc.vector.tensor_mul(_ap(wv, 4 * q, [[NB, DT], [1, 4]]), g0q,
                                 _ap(vT, 4 * q, [[NB, DT], [1, 4]]))
            nc.vector.tensor_reduce(ndp[:, :, q, 1],
                                    _ap(wv, 4 * q, [[NB, DT], [1, 4]]),
                                    mybir.AxisListType.X, ALU.add)

        offs = SCHED_OFFS
        stages = list(zip(offs, (stA, stB, stC, stD, stE1, stE2, stF, stG,
                                 stH, stI, stJ, stK)))
        sched = sorted(((q * SCHED_DQ + off, q, fn) for q in range(4)
                        for off, fn in stages), key=lambda x: x[0])
        for _, q, fn in sched:
            fn(q)

        # ---- tail: block-level mSA row-0 softmax, gating, fusion ----
        nd0 = const.tile([128, DT, 4], F32, tag="nd0")
        nc.vector.tensor_reduce(nd0[:, :, 0],
                                _ap(ndp, 0, [[8, DT], [2, 4]]),
                                mybir.AxisListType.X, ALU.add)
        nc.vector.tensor_reduce(nd0[:, :, 1],
                                _ap(ndp, 1, [[8, DT], [2, 4]]),
                                mybir.AxisListType.X, ALU.add)
        nc.vector.reciprocal(nd0[:, :, 2], nd0[:, :, 0])
        # o01[:, mt, {0,1}] = block-mSA rows {0, 15}; row 15 is 0
        o01h = const.tile([128, DT, 2], F16, tag="o01h")
        nc.vector.memset(o01h[:], 0.0)
        nc.vector.tensor_mul(o01h[:, :, 0], nd0[:, :, 1], nd0[:, :, 2])
        v01h = const.tile([128, DT, 2], F16, tag="v01h")
        nc.vector.tensor_copy(v01h[:], _ap(vT, 0, [[NB, DT], [NB - 1, 2]]))
        # G = 0.5 + 0.5*tanh(z/2);  e01 = v + (tanh+1)*d, d = 0.5*(o - v)
        pg = psml.tile([128, DT, 2], F32, tag="sml")
        for mt in range(DT):
            for kt in range(DT):
                nc.tensor.matmul(
                    pg[:, mt, :],
                    wp["gW1"][:, kt, mt * 128:(mt + 1) * 128],
                    o01h[:, kt, :], start=(kt == 0), stop=False)
            for kt in range(DT):
                nc.tensor.matmul(
                    pg[:, mt, :],
                    wp["gW2"][:, kt, mt * 128:(mt + 1) * 128],
                    v01h[:, kt, :], start=False, stop=(kt == DT - 1))
        tgs = const.tile([128, DT, 2], F32, tag="tgs")
        for mt in range(DT):
            nc.vector.tensor_scalar(tgs[:, mt, :], pg[:, mt, :], 0.5,
                                    bsb["gbh"][:, mt:mt + 1], ALU.mult,
                                    ALU.add)
        tg = const.tile([128, DT, 2], F16, tag="tg")
        nc.scalar.activation(tg[:], tgs[:], AF.Tanh)
        dg = const.tile([128, DT, 2], F16, tag="dg")
        e01h = const.tile([128, DT, 2], F16, tag="e01h")
        nc.vector.tensor_sub(dg[:], o01h[:], v01h[:])
        nc.vector.tensor_scalar_mul(dg[:], dg[:], 0.5)
        nc.vector.scalar_tensor_tensor(e01h[:], tg[:], 1.0, dg[:],
                                       ALU.add, ALU.mult)
        nc.vector.tensor_add(e01h[:], e01h[:], v01h[:])
        fus = const.tile([128, DT, 32], F16, tag="fus")
        tf = const.tile([128, DT, 32], F16, tag="tf")
        fstg = const.tile([128, DT, 32], F32, tag="fstg")
        fstg2 = const.tile([128, DT, 32], F32, tag="fstg2")
        for (wname, bname, dst, stg) in (
                ("fW1", "fb1h", fus, fstg),
                ("fW2", "fb2h", tf, fstg2)):
            pt = psml.tile([128, DT, 32], F32, tag="sml")
            for mt in range(DT):
                for kt in range(6):
                    if kt < 2:
                        rhs = _ap(inp, kt * T, [[T - 16, 2], [1, 16]])
                    elif kt < 4:
                        rhs = _ap(hT, (kt - 2) * T, [[T - 16, 2], [1, 16]])
                    else:
                        rhs = _ap(e01h, (kt - 4) * 2, [[1, 2], [0, 16]])
                    nc.tensor.matmul(
                        pt[:, mt, :],
                        wp[wname][:, kt, mt * 128:(mt + 1) * 128],
                        rhs, start=(kt == 0), stop=(kt == 5))
                nc.vector.tensor_scalar(stg[:, mt, :], pt[:, mt, :], 0.5,
                                        bsb[bname][:, mt:mt + 1], ALU.mult,
                                        ALU.add)
            # relu is homogeneous: fus is computed pre-halved (0.5*fb1 too)
            nc.scalar.activation(dst[:], stg[:],
                                 AF.Relu if dst is fus else AF.Tanh)

        # fus/xf16 are pre-halved, so: out = (tf+1)*fus - (tf-1)*xf16
        xf_ap = _ap(inp, 0, [[T, DT], [T - 16, 2], [1, 16]])
        xf16 = const.tile([128, DT, 32], F16, tag="xf16")
        nc.scalar.activation(xf16[:], xf_ap, AF.Copy, scale=0.5)
        sa = const.tile([128, DT, 32], F16, tag="sa")
        sb = const.tile([128, DT, 32], F16, tag="sb")
        outT = const.tile([128, DT, 32], F16, tag="outT")
        nc.vector.scalar_tensor_tensor(sa[:], tf[:], 1.0, fus[:],
                                       ALU.add, ALU.mult)
        nc.vector.scalar_tensor_tensor(sb[:], tf[:], 1.0, xf16[:],
                                       ALU.subtract, ALU.mult)
        nc.vector.tensor_sub(outT[:], sa[:], sb[:])
        nc.sync.dma_start(
            out=bass.AP(tensor=out_d.tensor, offset=0,
                        ap=[[32, 128], [128 * 32, DT], [1, 32]]),
            in_=outT[:])
    nc.compile()
    return nc


_NC = None


def _get_nc():
    global _NC
    if _NC is None:
        _NC = build_nc()
    return _NC


def _kt_pack(w):
    """[K, E] -> [128, (kt e)] matching rearrange('(kt p) e -> p kt e')."""
    kt = w.shape[0] // 128
    return np.transpose(w.reshape(kt, 128, -1), (1, 0, 2)).reshape(128, -1)


def _consts():
    p = np.arange(128)
    jj = p[:, None]
    ii = p[None, :]
    tri = ((jj // 64 == ii // 64) & (jj % 64 > ii % 64)).astype(np.float32)
    e63 = ((jj == ii) & (ii % 64 == 63)).astype(np.float32)
    idm = np.eye(128, dtype=np.float16)
    bks = []
    for t in range(2):
        b = np.zeros((128, 4), np.float16)
        b[np.arange(128), 2 * t + (np.arange(128) // 64)] = 1.0
        bks.append(b)
    mask0 = np.broadcast_to((np.arange(NB) > 0).astype(np.float16), (128, NB))
    return tri, e63, idm, bks, mask0


def prep_in_maps(inputs):
    x = np.asarray(inputs["x"], np.float32)
    tri, e63, idm, bks, mask0 = _consts()
    in_maps = []
    for core in range(NCORES):
        b = core % B
        sfx = "_fw" if core < B else "_bw"
        xf = x[b].reshape(T, D)
        if core >= B:
            xf = xf[::-1]

        w = {nm: np.asarray(inputs[nm + sfx], np.float32)
             for nm in ("fcW", "mW1", "mW2", "s2tW1", "s2tW", "gW1", "gW2",
                        "fW1", "fW2")}
        bv = {nm: np.asarray(inputs[nm + sfx], np.float32)
              for nm in ("fcb", "mb", "s2tb1", "s2tb", "gb", "fb1", "fb2")}

        packh = np.zeros((128, NPACKH), np.float16)
        for nm in ("fcW", "mW1", "mW2", "s2tW1", "s2tW", "gW1", "gW2",
                   "fW1", "fW2"):
            kp = _kt_pack(w[nm]).astype(np.float16)
            packh[:, PH[nm]:PH[nm] + kp.shape[1]] = kp
        packh[:, PH["triC1"]:PH["triC1"] + 128] = (C1 * tri).astype(np.float16)
        packh[:, PH["triC1E"]:PH["triC1E"] + 128] = \
            (C1 * tri + e63).astype(np.float16)
        packh[:, PH["triC2"]:PH["triC2"] + 128] = (C2 * tri).astype(np.float16)
        packh[:, PH["idm"]:PH["idm"] + 128] = idm
        packh[:, PH["bk0"]:PH["bk0"] + 4] = bks[0]
        packh[:, PH["bk1"]:PH["bk1"] + 4] = bks[1]
        packh[:, PH["mask0"]:PH["mask0"] + NB] = mask0
        packh[0, PH["ones_row"]:PH["ones_row"] + 256] = 1.0
        packh[0, PH["fcbT"]:PH["fcbT"] + D] = bv["fcb"]
        packh[0, PH["mb_row"]:PH["mb_row"] + D] = bv["mb"]
        packh[0, PH["s2tb_row"]:PH["s2tb_row"] + D] = bv["s2tb"]

        packa = np.zeros((128, 15), np.float32)
        for nm, src, scl in (("fcb", "fcb", 1.0), ("s2tb1", "s2tb1", 1.0),
                             ("gbh", "gb", 0.5), ("fb1h", "fb1", 0.5),
                             ("fb2h", "fb2", 0.5), ("mbf", "mb", 0.2)):
            packa[:, PB[nm]:PB[nm] + DT] = (scl * bv[src]).reshape(DT, 128).T
        packa[:, P_BZ] = -SFIT * SHIFT
        packa[:, P_BW] = BW
        packa[:, P_BZ4] = -2.0 * SFIT * SHIFT
        packh[:, PH["bias"]:PH["bias"] + 30] = packa.view(np.float16)

        m = {"xT": np.ascontiguousarray(xf.T).astype(np.float16),
             "packf16": packh}
        in_maps.append(m)
    return in_maps


def assemble(outs):
    u_fw = np.stack([outs[b]["outT"][:, 0:16].T for b in range(B)])
    u_bw = np.stack([outs[B + b]["outT"][:, 16:32].T[::-1] for b in range(B)])
    return np.concatenate([u_fw, u_bw], axis=-1).astype(np.float32)


def kernel(**inputs):
    in_maps = prep_in_maps(inputs)
    res = bass_utils.run_bass_kernel_spmd(_get_nc(), in_maps,
                                          core_ids=list(range(NCORES)))
    return assemble(res.results)


# revision 103
# speedup vs baseline: 1.0000x; 1.0000x over previous
"""BiBloSAN Trainium2 kernel — rank-2 separable softmax approximation.

Shapes: B=4, N=16 blocks, R=64 tokens/block, D=256.
Sharding: one (batch, direction) pair per core -> 8 cores, no collectives.
The bw direction runs the SAME SPMD program on a host-reversed token
sequence (flat reverse maps the j<i mask onto the j>i program exactly).

Intra-block mSA approximation: the pairwise weight
    g(u) = exp(C*tanh(u/C)),  u = xi[i,d] + xj[j,d] + b[d]
is replaced by a 2-term exponential fit
    g(u) ~= c1 e^{s u} + c2 e^{2 s u}
tuned END-TO-END against the exact reference (max rel err 3.8e-3 in a
bit-accurate numpy mirror; gate is 2e-2).  Each term is separable:
e^{ksu} = (zh wh)^{2k} with zh = e^{(s/2)(xjb-SH)}, wh = e^{(s/2)(xi+SH)},
so the masked-softmax num/den become per-block suffix sums of zh-powers
(triangular matmuls, c_k folded into the stationary).  The common factor
wh^2 cancels in num/den, so the recombination is a single Horner step:
    num|den = (wh^2 ⊙ S2) + S1,   h = num/den
where S1 = c1·tri @ [z^2 x | z^2] (den stationary carries an extra
diagonal at the last row of each block so empty rows give h=0), and
S2 = c2·tri @ [z^4 x | z^4].

s2t block summaries are computed token-major so the per-block softmax
sums become matmuls against block-indicator stationaries (no DVE
reductions).  Sigmoids are rewritten as 0.5+0.5*tanh(z/2) to stay on the
exp/tanh/relu activation table (no table reloads).
"""

import numpy as np
from contextlib import ExitStack

import concourse.bass as bass
import concourse.mybir as mybir
import concourse.tile as tile
from concourse import bacc, bass_utils

F32 = mybir.dt.float32
F16 = mybir.dt.float16
AF = mybir.ActivationFunctionType
ALU = mybir.AluOpType

B, NB, R, D = 4, 16, 64, 256
T = NB * R          # 1024 tokens
DT = D // 128       # 2 partition tiles of feature dim
NCORES = 8
NTILE = T // 128    # 8 token tiles (2 blocks each)

# end-to-end tuned rank-2 fit of exp(5*tanh(u/5)):
#   g(u) ~= C1 e^{S u} + C2 e^{2 S u}
SFIT = 0.97664077
C1 = 0.76476878
C2 = -0.00151352
SHIFT = 2.0
S2F = SFIT / 2.0
BZ = -S2F * SHIFT   # zh = exp(S2F*xjb + BZ)
BW = SFIT * SHIFT   # w2 = exp(SFIT*xi + BW)

# software-pipeline schedule (stage offsets in us-equivalents, quarter pitch)
SCHED_OFFS = (0.0, 1.065, 2.047, 2.331, 2.862, 2.883, 4.286, 4.45,
              5.321, 6.457, 6.833, 7.519)
SCHED_DQ = 0.95
WORK_BUFS = 4
PGEM_BUFS = 3
PSP_BUFS = 2
PSML_BUFS = 3

# f16 pack column offsets
PH = {}
_c = 0
def _ph(nm, w):
    global _c
    PH[nm] = _c
    _c += w
_ph("bias", 30)     # f32 per-partition biases, bitcast into the f16 pack
_ph("fcW", 512)
_ph("fcbT", D)
_ph("idm", 128)
NPK0 = _c           # DMA chunk 0: stage A only
_ph("mW2", 512)
_ph("mb_row", D)
NPK1 = _c           # DMA chunk 1: stage B (xjb)
_ph("triC1", 128)
_ph("triC1E", 128)
_ph("triC2", 128)
_ph("mW1", 512)
NPK2 = _c           # DMA chunk 2: stages B-E
_ph("bk0", 4)       # block indicator, tile 0 of quarter
_ph("bk1", 4)
_ph("mask0", NB)
_ph("ones_row", 256)
_ph("s2tb_row", D)
_ph("s2tW1", 512)
_ph("s2tW", 512)
NPK3 = _c           # DMA chunk 3: stages G-I
_ph("gW1", 512)
_ph("gW2", 512)
_ph("fW1", 1536)
_ph("fW2", 1536)
NPACKH = _c

# f32 per-partition bias columns (feature-major, DT cols each) inside the
# bitcast "bias" block of the f16 pack
PB = {"fcb": 0, "s2tb1": 2, "gbh": 4, "fb1h": 6, "fb2h": 8, "mbf": 10}
P_BZ, P_BW, P_BZ4 = 12, 13, 14  # broadcast scalar biases for the exps


def _ap(t, offset, dims):
    """Raw AP on sbuf/psum tile t: dims = [[step, count], ...] free dims."""
    base = t[:]
    return bass.AP(tensor=base.tensor, offset=base.offset + offset,
                   ap=[list(base.ap[0])] + [list(d) for d in dims])


def build_nc():
    nc = bacc.Bacc("TRN2", target_bir_lowering=False, debug=False,
                   num_devices=NCORES)

    xT_d = nc.dram_tensor("xT", [D, T], F16, kind="ExternalInput").ap()
    packh_d = nc.dram_tensor("packf16", [128, NPACKH], F16,
                             kind="ExternalInput").ap()
    out_d = nc.dram_tensor("outT", [D, 32], F16, kind="ExternalOutput").ap()

    with tile.TileContext(nc) as tc, ExitStack() as ctx:
        ctx.enter_context(nc.allow_low_precision(
            reason="f16 softmax pipeline validated end-to-end vs reference"))
        # noqa: engine split: Act=exps/relus (PSUM-fed), DVE=PSUM-touching
        # muls/recips, Pool(gpsimd)=SBUF-only muls, PE=GEMMs+suffix-sums
        const = ctx.enter_context(tc.tile_pool(name="const", bufs=1))
        big = ctx.enter_context(tc.tile_pool(name="big", bufs=1))
        work = ctx.enter_context(tc.tile_pool(name="work", bufs=WORK_BUFS))
        pgem = ctx.enter_context(
            tc.tile_pool(name="pgem", bufs=PGEM_BUFS, space="PSUM"))
        psp = ctx.enter_context(
            tc.tile_pool(name="psp", bufs=PSP_BUFS, space="PSUM"))
        psml = ctx.enter_context(
            tc.tile_pool(name="psml", bufs=PSML_BUFS, space="PSUM"))

        # ---- DMA loads, ordered by consumer stage ----
        pkh = const.tile([128, NPACKH], F16, tag="packh")
        xT = big.tile([128, DT, T], F16, tag="xT")
        nc.sync.dma_start(out=pkh[:, 0:NPK0], in_=packh_d[:, 0:NPK0])
        nc.sync.dma_start(
            out=_ap(xT, 0, [[T, DT], [1, 512]]),
            in_=bass.AP(tensor=xT_d.tensor, offset=0,
                        ap=[[T, 128], [128 * T, DT], [1, 512]]))
        nc.sync.dma_start(out=pkh[:, NPK0:NPK2], in_=packh_d[:, NPK0:NPK2])
        nc.sync.dma_start(
            out=_ap(xT, 512, [[T, DT], [1, 512]]),
            in_=bass.AP(tensor=xT_d.tensor, offset=512,
                        ap=[[T, 128], [128 * T, DT], [1, 512]]))
        nc.sync.dma_start(out=pkh[:, NPK2:NPK3], in_=packh_d[:, NPK2:NPK3])
        nc.sync.dma_start(out=pkh[:, NPK3:], in_=packh_d[:, NPK3:])

        wp = {nm: pkh[:, c:c + 512].rearrange("p (kt e) -> p kt e", kt=DT)
              for nm, c in PH.items()
              if nm in ("fcW", "mW1", "mW2", "s2tW1", "s2tW", "gW1", "gW2")}
        wp.update({nm: pkh[:, PH[nm]:PH[nm] + 1536].rearrange(
            "p (kt e) -> p kt e", kt=6) for nm in ("fW1", "fW2")})
        triC1 = pkh[:, PH["triC1"]:PH["triC1"] + 128]
        triC1E = pkh[:, PH["triC1E"]:PH["triC1E"] + 128]
        triC2 = pkh[:, PH["triC2"]:PH["triC2"] + 128]
        idm = pkh[:, PH["idm"]:PH["idm"] + 128]
        bk = [pkh[:, PH["bk0"]:PH["bk0"] + 4], pkh[:, PH["bk1"]:PH["bk1"] + 4]]
        mask0 = pkh[:, PH["mask0"]:PH["mask0"] + NB]
        ones_row = pkh[0:1, PH["ones_row"]:PH["ones_row"] + 128]
        ones_row256 = pkh[0:1, PH["ones_row"]:PH["ones_row"] + 256]
        mb_row = pkh[0:1, PH["mb_row"]:PH["mb_row"] + D]
        s2tb_row = pkh[0:1, PH["s2tb_row"]:PH["s2tb_row"] + D]
        bsb = {nm: pkh[:, 2 * c:2 * (c + DT)].bitcast(F32)
               for nm, c in PB.items()}

        # dummy activation to hoist the exp-table load off the critical path
        wrm = const.tile([1, 2], F32, tag="wrm")
        nc.vector.memset(wrm[:], 0.0)
        nc.scalar.activation(wrm[:, 1:2], wrm[:, 0:1], AF.Exp)

        inp = big.tile([128, DT, T], F16, tag="inp")
        h_tok = big.tile([128, NTILE, D], F16, tag="h_tok")
        hT = big.tile([128, DT, T], F16, tag="hT")
        v_sb = big.tile([4, 4, D], F16, tag="v_sb")
        vT = const.tile([128, DT, NB], F16, tag="vT")
        vi_sb = const.tile([128, DT, 1], F32, tag="vi_sb")
        u0 = const.tile([128, DT, NB], F32, tag="u0")
        g0 = const.tile([128, DT, NB], F16, tag="g0")
        wv = const.tile([128, DT, NB], F16, tag="wv")
        ndp = const.tile([128, DT, 4, 2], F32, tag="ndp")
        qs = [dict() for _ in range(4)]

        # The engines execute their queues IN PROGRAM ORDER, so the quarters
        # are emitted as software-pipelined stages, interleaved by expected
        # start time; otherwise quarter q+1's ready work would sit blocked
        # behind quarter q's unfinished chain in every engine queue.
        def stA(q):  # P1 chunk (feature-major FC) + inpH via transposes
            tok0 = q * 256
            p1 = pgem.tile([128, DT, 256], F32, tag="gem", name="p1")
            for mt in range(DT):
                for kt in range(DT):
                    nc.tensor.matmul(
                        p1[:, mt, :],
                        wp["fcW"][:, kt, mt * 128:(mt + 1) * 128],
                        xT[:, kt, tok0:tok0 + 256],
                        start=(kt == 0), stop=False)
                nc.tensor.matmul(
                    p1[:, mt, :],
                    pkh[0:1, PH["fcbT"] + mt * 128:PH["fcbT"] + mt * 128 + 128],
                    ones_row256, start=False, stop=True)
            nc.scalar.activation(
                _ap(inp, tok0, [[T, DT], [1, 256]]), p1[:], AF.Relu)
            ptr2 = psml.tile([128, 2, DT, 128], F16, tag="sml", name="ptr2")
            for t in range(2):
                tk = tok0 + t * 128
                for dt in range(DT):
                    nc.tensor.transpose(ptr2[:, t, dt, :],
                                        inp[:, dt, tk:tk + 128], idm)
            qs[q]["ptr2"] = ptr2

        def stB(q):  # xi/xjb GEMMs
            tok0 = q * 256
            pxi = pgem.tile([128, 2, D], F32, tag="gem", name="pxi")
            pxj = pgem.tile([128, 2, D], F32, tag="gem", name="pxj")
            for t in range(2):
                tk = tok0 + t * 128
                for kt in range(DT):
                    nc.tensor.matmul(pxj[:, t, :], inp[:, kt, tk:tk + 128],
                                     wp["mW2"][:, kt, :],
                                     start=(kt == 0), stop=False)
                nc.tensor.matmul(pxj[:, t, :], ones_row, mb_row,
                                 start=False, stop=True)
                for kt in range(DT):
                    nc.tensor.matmul(pxi[:, t, :], inp[:, kt, tk:tk + 128],
                                     wp["mW1"][:, kt, :],
                                     start=(kt == 0), stop=(kt == DT - 1))
            qs[q]["pxi"], qs[q]["pxj"] = pxi, pxj

        def stC(q):  # exps: z2 (from pxj), w2 (from pxi)
            pxi, pxj = qs[q]["pxi"], qs[q]["pxj"]
            w2 = work.tile([128, 2, D], F32, tag="w2", name="w2")
            zall = work.tile([128, 2, 2, 2, D], F16, tag="zall", name="zall")
            nc.scalar.activation(
                _ap(zall, 2 * D, [[4 * D, 2], [1, D]]), pxj[:],
                AF.Exp, scale=SFIT,
                bias=pkh[:, 2 * P_BZ:2 * P_BZ + 2].bitcast(F32))    # z2
            nc.scalar.activation(w2[:], pxi[:], AF.Exp, scale=SFIT,
                                 bias=pkh[:, 2 * P_BW:2 * P_BW + 2]
                                 .bitcast(F32))
            qs[q]["w2"], qs[q]["zall"] = w2, zall

        def stD(q):  # z powers: zall[p,t,xp,k,d]: xp=0 -> z^k*x, xp=1 -> z^k
            zall = qs[q]["zall"]
            ptr2 = qs[q]["ptr2"]
            inpH_b = _ap(ptr2, 0, [[D, 2], [1, D]])
            nc.gpsimd.tensor_tensor(
                _ap(zall, 3 * D, [[4 * D, 2], [1, D]]),
                _ap(zall, 2 * D, [[4 * D, 2], [1, D]]),
                _ap(zall, 2 * D, [[4 * D, 2], [1, D]]), ALU.mult)      # z4
            nc.vector.tensor_mul(
                _ap(zall, 0, [[4 * D, 2], [1, D]]),
                _ap(zall, 2 * D, [[4 * D, 2], [1, D]]),
                inpH_b)                                                # z2*x
            nc.vector.tensor_mul(
                _ap(zall, 1 * D, [[4 * D, 2], [1, D]]),
                _ap(zall, 3 * D, [[4 * D, 2], [1, D]]), inpH_b)       # z4*x

        def stE1(q):  # S2 suffix-sum matmuls + in-place w2 mul
            zall, w2 = qs[q]["zall"], qs[q]["w2"]
            Ss = []
            for t in range(2):
                S = psp.tile([128, 2 * D], F32, tag="S", name="S")
                nc.tensor.matmul(S[:], triC2,
                                 _ap(zall, t * 4 * D + D,
                                     [[2 * D, 2], [1, D]]),
                                 start=True, stop=True)
                Ss.append(S)
            for t in range(2):
                w2_b = _ap(w2, t * D, [[0, 2], [1, D]])
                nc.vector.tensor_mul(Ss[t][:], Ss[t][:], w2_b)
            qs[q]["Ss"] = Ss

        def stE2(q):  # S1 matmuls ACCUMULATE on top in PSUM, then h = n/d
            zall, Ss = qs[q]["zall"], qs[q]["Ss"]
            for t in range(2):
                nc.tensor.matmul(Ss[t][:, 0:D], triC1,
                                 _ap(zall, t * 4 * D, [[1, D]]),
                                 start=False, stop=True,
                                 skip_group_check=True)
                nc.tensor.matmul(Ss[t][:, D:2 * D], triC1E,
                                 _ap(zall, t * 4 * D + 2 * D, [[1, D]]),
                                 start=False, stop=True,
                                 skip_group_check=True)
            rden = work.tile([128, 2, D], F32, tag="rden", name="rden")
            for t in range(2):
                nc.vector.reciprocal(rden[:, t, :], Ss[t][:, D:2 * D])
                nc.vector.tensor_mul(h_tok[:, 2 * q + t, :], Ss[t][:, 0:D],
                                     rden[:, t, :])

        def stF(q):  # transpose h to feature-major
            for t in range(2):
                ptr = psml.tile([128, DT, 128], F16, tag="sml", name="ptr")
                for dt in range(DT):
                    nc.tensor.transpose(
                        ptr[:, dt, :],
                        h_tok[:, 2 * q + t, dt * 128:(dt + 1) * 128], idm)
                hT_dst = _ap(hT, (2 * q + t) * 128, [[T, DT], [1, 128]])
                nc.scalar.activation(hT_dst, ptr[:], AF.Copy)

        def stG(q):  # s2t stage 1: f = relu(hT.T@W1 + b1), feature-major
            tok0 = q * 256
            pf = psml.tile([128, DT, 256], F32, tag="sml", name="pf")
            for t in range(2):
                for mt in range(DT):
                    for kt in range(DT):
                        nc.tensor.matmul(
                            pf[:, mt, t * 128:(t + 1) * 128],
                            wp["s2tW1"][:, kt, mt * 128:(mt + 1) * 128],
                            hT[:, kt, tok0 + t * 128:tok0 + (t + 1) * 128],
                            start=(kt == 0), stop=(kt == DT - 1))
            fTq = work.tile([128, DT, 256], F16, tag="fTq", name="fTq")
            for mt in range(DT):
                nc.scalar.activation(fTq[:, mt, :], pf[:, mt, :], AF.Relu,
                                     bias=bsb["s2tb1"][:, mt:mt + 1])
            qs[q]["fTq"] = fTq

        def stH(q):  # s2t stage 2: e = exp(f@W + b), token-major
            fTq = qs[q]["fTq"]
            end = work.tile([128, 2, 2, D], F16, tag="end", name="end")
            pe = psml.tile([128, 2, D], F32, tag="sml", name="pe")
            for t in range(2):
                for mt in range(DT):
                    nc.tensor.matmul(pe[:, t, :],
                                     fTq[:, mt, t * 128:(t + 1) * 128],
                                     wp["s2tW"][:, mt, :],
                                     start=(mt == 0), stop=False)
                nc.tensor.matmul(pe[:, t, :], ones_row, s2tb_row,
                                 start=False, stop=True)
            nc.scalar.activation(_ap(end, 0, [[2 * D, 2], [1, D]]), pe[:],
                                 AF.Exp)
            qs[q]["end"] = end

        def stI(q):  # block sums via indicator matmuls -> v rows
            end = qs[q]["end"]
            vq = psml.tile([4, 2 * D], F32, tag="sml", name="vq")
            for t in range(2):
                nc.gpsimd.tensor_tensor(end[:, t, 1, :], end[:, t, 0, :],
                                        h_tok[:, 2 * q + t, :], ALU.mult)
                nc.tensor.matmul(vq[:], bk[t], end[:, t, :, :],
                                 start=(t == 0), stop=(t == 1))
            rdv = work.tile([4, D], F32, tag="rdv", name="rdv")
            nc.vector.reciprocal(rdv[:], vq[:, 0:D])
            nc.vector.tensor_mul(v_sb[:, q, :], vq[:, D:2 * D], rdv[:])

        def stJ(q):  # transpose this quarter's 4 v rows into vT columns
            ptrV = psml.tile([128, DT, 4], F16, tag="sml", name="ptrV")
            for dt in range(DT):
                nc.tensor.transpose(
                    ptrV[:, dt, :], v_sb[:, q, dt * 128:(dt + 1) * 128],
                    pkh[0:4, PH["idm"]:PH["idm"] + 4])
            nc.vector.tensor_copy(vT[:, :, 4 * q:4 * q + 4], ptrV[:])

        def stK(q):  # block-mSA row-0 partials for this quarter's 4 columns
            pvj = psml.tile([128, DT, 4], F32, tag="sml", name="pvj")
            if q == 0:
                pvi = psml.tile([128, DT, 1], F32, tag="sml", name="pvi")
                for mt in range(DT):
                    for kt in range(DT):
                        nc.tensor.matmul(
                            pvi[:, mt, :],
                            wp["mW1"][:, kt, mt * 128:(mt + 1) * 128],
                            vT[:, kt, 0:1], start=(kt == 0),
                            stop=(kt == DT - 1))
                nc.vector.tensor_copy(vi_sb[:], pvi[:])
            for mt in range(DT):
                for kt in range(DT):
                    nc.tensor.matmul(
                        pvj[:, mt, :],
                        wp["mW2"][:, kt, mt * 128:(mt + 1) * 128],
                        vT[:, kt, 4 * q:4 * q + 4], start=(kt == 0),
                        stop=(kt == DT - 1))
            for mt in range(DT):
                nc.vector.scalar_tensor_tensor(
                    u0[:, mt, 4 * q:4 * q + 4], pvj[:, mt, :],
                    bsb["mbf"][:, mt:mt + 1],
                    _ap(vi_sb, mt, [[0, 4]]), ALU.add, ALU.add)
            u0q = _ap(u0, 4 * q, [[NB, DT], [1, 4]])
            g0q = _ap(g0, 4 * q, [[NB, DT], [1, 4]])
            nc.scalar.activation(u0q, u0q, AF.Tanh, scale=0.2)
            nc.scalar.activation(g0q, u0q, AF.Exp, scale=5.0)
            if q == 0:
                nc.vector.memset(_ap(g0, 0, [[NB, DT], [1, 1]]), 0.0)
            nc.vector.tensor_reduce(ndp[:, :, q, 0], g0q,
                                    mybir.AxisListType.X, ALU.add)
            nc.vector.tensor_mul(_ap(wv, 4 * q, [[NB, DT], [1, 4]]), g0q,
                                 _ap(vT, 4 * q, [[NB, DT], [1, 4]]))
            nc.vector.tensor_reduce(ndp[:, :, q, 1],
                                    _ap(wv, 4 * q, [[NB, DT], [1, 4]]),
                                    mybir.AxisListType.X, ALU.add)

        offs = SCHED_OFFS
        stages = list(zip(offs, (stA, stB, stC, stD, stE1, stE2, stF, stG,
                                 stH, stI, stJ, stK)))
        sched = sorted(((q * SCHED_DQ + off, q, fn) for q in range(4)
                        for off, fn in stages), key=lambda x: x[0])
        for _, q, fn in sched:
            fn(q)

        # ---- tail: block-level mSA row-0 softmax, gating, fusion ----
        nd0 = const.tile([128, DT, 4], F32, tag="nd0")
        nc.vector.tensor_reduce(nd0[:, :, 0],
                                _ap(ndp, 0, [[8, DT], [2, 4]]),
                                mybir.AxisListType.X, ALU.add)
        nc.vector.tensor_reduce(nd0[:, :, 1],
                                _ap(ndp, 1, [[8, DT], [2, 4]]),
                                mybir.AxisListType.X, ALU.add)
        nc.vector.reciprocal(nd0[:, :, 2], nd0[:, :, 0])
        # o01[:, mt, {0,1}] = block-mSA rows {0, 15}; row 15 is 0
        o01h = const.tile([128, DT, 2], F16, tag="o01h")
        nc.vector.memset(o01h[:], 0.0)
        nc.vector.tensor_mul(o01h[:, :, 0], nd0[:, :, 1], nd0[:, :, 2])
        v01h = const.tile([128, DT, 2], F16, tag="v01h")
        nc.vector.tensor_copy(v01h[:], _ap(vT, 0, [[NB, DT], [NB - 1, 2]]))
        # G = 0.5 + 0.5*tanh(z/2);  e01 = v + (tanh+1)*d, d = 0.5*(o - v)
        pg = psml.tile([128, DT, 2], F32, tag="sml")
        for mt in range(DT):
            for kt in range(DT):
                nc.tensor.matmul(
                    pg[:, mt, :],
                    wp["gW1"][:, kt, mt * 128:(mt + 1) * 128],
                    o01h[:, kt, :], start=(kt == 0), stop=False)
            for kt in range(DT):
                nc.tensor.matmul(
                    pg[:, mt, :],
                    wp["gW2"][:, kt, mt * 128:(mt + 1) * 128],
                    v01h[:, kt, :], start=False, stop=(kt == DT - 1))
        tgs = const.tile([128, DT, 2], F32, tag="tgs")
        for mt in range(DT):
            nc.vector.tensor_scalar(tgs[:, mt, :], pg[:, mt, :], 0.5,
                                    bsb["gbh"][:, mt:mt + 1], ALU.mult,
                                    ALU.add)
        tg = const.tile([128, DT, 2], F16, tag="tg")
        nc.scalar.activation(tg[:], tgs[:], AF.Tanh)
        dg = const.tile([128, DT, 2], F16, tag="dg")
        e01h = const.tile([128, DT, 2], F16, tag="e01h")
        nc.vector.tensor_sub(dg[:], o01h[:], v01h[:])
        nc.vector.tensor_scalar_mul(dg[:], dg[:], 0.5)
        nc.vector.scalar_tensor_tensor(e01h[:], tg[:], 1.0, dg[:],
                                       ALU.add, ALU.mult)
        nc.vector.tensor_add(e01h[:], e01h[:], v01h[:])
        fus = const.tile([128, DT, 32], F16, tag="fus")
        tf = const.tile([128, DT, 32], F16, tag="tf")
        fstg = const.tile([128, DT, 32], F32, tag="fstg")
        fstg2 = const.tile([128, DT, 32], F32, tag="fstg2")
        for (wname, bname, dst, stg) in (
                ("fW1", "fb1h", fus, fstg),
                ("fW2", "fb2h", tf, fstg2)):
            pt = psml.tile([128, DT, 32], F32, tag="sml")
            for mt in range(DT):
                for kt in range(6):
                    if kt < 2:
                        rhs = _ap(inp, kt * T, [[T - 16, 2], [1, 16]])
                    elif kt < 4:
                        rhs = _ap(hT, (kt - 2) * T, [[T - 16, 2], [1, 16]])
                    else:
                        rhs = _ap(e01h, (kt - 4) * 2, [[1, 2], [0, 16]])
                    nc.tensor.matmul(
                        pt[:, mt, :],
                        wp[wname][:, kt, mt * 128:(mt + 1) * 128],
                        rhs, start=(kt == 0), stop=(kt == 5))
                nc.vector.tensor_scalar(stg[:, mt, :], pt[:, mt, :], 0.5,
                                        bsb[bname][:, mt:mt + 1], ALU.mult,
                                        ALU.add)
            # relu is homogeneous: fus is computed pre-halved (0.5*fb1 too)
            nc.scalar.activation(dst[:], stg[:],
                                 AF.Relu if dst is fus else AF.Tanh)

        # fus/xf16 are pre-halved, so: out = (tf+1)*fus - (tf-1)*xf16
        xf_ap = _ap(inp, 0, [[T, DT], [T - 16, 2], [1, 16]])
        xf16 = const.tile([128, DT, 32], F16, tag="xf16")
        nc.scalar.activation(xf16[:], xf_ap, AF.Copy, scale=0.5)
        sa = const.tile([128, DT, 32], F16, tag="sa")
        sb = const.tile([128, DT, 32], F16, tag="sb")
        outT = const.tile([128, DT, 32], F16, tag="outT")
        nc.vector.scalar_tensor_tensor(sa[:], tf[:], 1.0, fus[:],
                                       ALU.add, ALU.mult)
        nc.vector.scalar_tensor_tensor(sb[:], tf[:], 1.0, xf16[:],
                                       ALU.subtract, ALU.mult)
        nc.vector.tensor_sub(outT[:], sa[:], sb[:])
        nc.sync.dma_start(
            out=bass.AP(tensor=out_d.tensor, offset=0,
                        ap=[[32, 128], [128 * 32, DT], [1, 32]]),
            in_=outT[:])
    nc.compile()
    return nc


_NC = None


def _get_nc():
    global _NC
    if _NC is None:
        _NC = build_nc()
    return _NC


def _kt_pack(w):
    """[K, E] -> [128, (kt e)] matching rearrange('(kt p) e -> p kt e')."""
    kt = w.shape[0] // 128
    return np.transpose(w.reshape(kt, 128, -1), (1, 0, 2)).reshape(128, -1)


def _consts():
    p = np.arange(128)
    jj = p[:, None]
    ii = p[None, :]
    tri = ((jj // 64 == ii // 64) & (jj % 64 > ii % 64)).astype(np.float32)
    e63 = ((jj == ii) & (ii % 64 == 63)).astype(np.float32)
    idm = np.eye(128, dtype=np.float16)
    bks = []
    for t in range(2):
        b = np.zeros((128, 4), np.float16)
        b[np.arange(128), 2 * t + (np.arange(128) // 64)] = 1.0
        bks.append(b)
    mask0 = np.broadcast_to((np.arange(NB) > 0).astype(np.float16), (128, NB))
    return tri, e63, idm, bks, mask0


def prep_in_maps(inputs):
    x = np.asarray(inputs["x"], np.float32)
    tri, e63, idm, bks, mask0 = _consts()
    in_maps = []
    for core in range(NCORES):
        b = core % B
        sfx = "_fw" if core < B else "_bw"
        xf = x[b].reshape(T, D)
        if core >= B:
            xf = xf[::-1]

        w = {nm: np.asarray(inputs[nm + sfx], np.float32)
             for nm in ("fcW", "mW1", "mW2", "s2tW1", "s2tW", "gW1", "gW2",
                        "fW1", "fW2")}
        bv = {nm: np.asarray(inputs[nm + sfx], np.float32)
              for nm in ("fcb", "mb", "s2tb1", "s2tb", "gb", "fb1", "fb2")}

        packh = np.zeros((128, NPACKH), np.float16)
        for nm in ("fcW", "mW1", "mW2", "s2tW1", "s2tW", "gW1", "gW2",
                   "fW1", "fW2"):
            kp = _kt_pack(w[nm]).astype(np.float16)
            packh[:, PH[nm]:PH[nm] + kp.shape[1]] = kp
        packh[:, PH["triC1"]:PH["triC1"] + 128] = (C1 * tri).astype(np.float16)
        packh[:, PH["triC1E"]:PH["triC1E"] + 128] = \
            (C1 * tri + e63).astype(np.float16)
        packh[:, PH["triC2"]:PH["triC2"] + 128] = (C2 * tri).astype(np.float16)
        packh[:, PH["idm"]:PH["idm"] + 128] = idm
        packh[:, PH["bk0"]:PH["bk0"] + 4] = bks[0]
        packh[:, PH["bk1"]:PH["bk1"] + 4] = bks[1]
        packh[:, PH["mask0"]:PH["mask0"] + NB] = mask0
        packh[0, PH["ones_row"]:PH["ones_row"] + 256] = 1.0
        packh[0, PH["fcbT"]:PH["fcbT"] + D] = bv["fcb"]
        packh[0, PH["mb_row"]:PH["mb_row"] + D] = bv["mb"]
        packh[0, PH["s2tb_row"]:PH["s2tb_row"] + D] = bv["s2tb"]

        packa = np.zeros((128, 15), np.float32)
        for nm, src, scl in (("fcb", "fcb", 1.0), ("s2tb1", "s2tb1", 1.0),
                             ("gbh", "gb", 0.5), ("fb1h", "fb1", 0.5),
                             ("fb2h", "fb2", 0.5), ("mbf", "mb", 0.2)):
            packa[:, PB[nm]:PB[nm] + DT] = (scl * bv[src]).reshape(DT, 128).T
        packa[:, P_BZ] = -SFIT * SHIFT
        packa[:, P_BW] = BW
        packa[:, P_BZ4] = -2.0 * SFIT * SHIFT
        packh[:, PH["bias"]:PH["bias"] + 30] = packa.view(np.float16)

        m = {"xT": np.ascontiguousarray(xf.T).astype(np.float16),
             "packf16": packh}
        in_maps.append(m)
    return in_maps


def assemble(outs):
    u_fw = np.stack([outs[b]["outT"][:, 0:16].T for b in range(B)])
    u_bw = np.stack([outs[B + b]["outT"][:, 16:32].T[::-1] for b in range(B)])
    return np.concatenate([u_fw, u_bw], axis=-1).astype(np.float32)


def kernel(**inputs):
    in_maps = prep_in_maps(inputs)
    res = bass_utils.run_bass_kernel_spmd(_get_nc(), in_maps,
                                          core_ids=list(range(NCORES)))
    return assemble(res.results)
